# revision 19
# baseline (speedup 1.0000x reference)
"""Causal multi-head attention kernel for Trainium2 (8 NeuronCores), v3.

Problem: B=2, H=16, S=2048, D=64 causal attention (softmax over last axis).
Sharding: 32 (batch, head) pairs split 4-per-core across 8 cores; each core
computes its heads independently (no collectives).

Design notes (engine-balance oriented; HW shows the PE often runs at the cold
1.2 GHz p-state for duty-cycled matmul streams, and ACT costs ~300ns per
ACTIVATE on top of N/1.2GHz; measured ~88us vs the 134us v1 baseline):
  - Head PAIRS advance through k-blocks together.  The two heads' QK matmuls
    (contraction = d = 64) are emitted back-to-back with lhsT base partitions
    0 / 64, so the PE row-tiles them into concurrent array halves (2x).
  - exp is computed per merged piece [128, 2heads, 512] (N=1024/instr) and
    SPLIT between ACT (exact, activation) and DVE (Schraudolph one-op bit
    trick: tensor_scalar mult+add -> int16, bitcast bf16) by DVE_EXP share.
  - forward PV (stationary = P^T chunk, rhs = V-block||ones [128,65]) into
    65-wide per-q_sub PSUM accumulators; LDW-bound, p-state immune.
  - intra-block causal mask via GPSIMD tensor multiply (Pool engine).
  - normalize on DVE (reciprocal + per-partition scalar mul), output bf16 in
    [head, r, j, c] layout (q = j*128+r) so out-DMA descriptors are >=512B;
    host inverse-permutes and casts.
  - 4-pass q split per pair (4 q_subs per pass) so PSUM fits THREE st slots
    (3 x [128,2,512] = 6 banks) + accs ([128,4,65] per head = 2 banks); QK is
    emitted two strips ahead so the in-order PE queue never blocks ACT/DVE on
    the st ring (this was the dominant pipeline stall at 2 slots).
  - all input DMAs are hoisted to the top of the rep on the SP queue (outputs
    go on the gpsimd queue) so the next rep's loads prefetch during compute;
    input rings are 3 deep for a full rep of prefetch lead.
"""

import sys

if "/opt/trn_rl_repo" not in sys.path:
    sys.path.insert(0, "/opt/trn_rl_repo")

import numpy as np
import ml_dtypes

B, H, S, D = 2, 16, 2048, 64
N_CORES = 8
HEADS_PER_CORE = (B * H) // N_CORES  # 4
KB = S // 128  # 16 k-blocks per head

_BF16 = ml_dtypes.bfloat16

_built = {}

MODE = "full"
_MODE_FLAGS = {
    "full": "dqepn",
    "qk_noio": "q",
    "exp_pure": "e",
    "qe_pure": "qe",
    "qep_pure": "qep",
    "no_out": "dqep",
}

# Fraction of exp columns computed on DVE (Schraudolph) instead of ACT.
DVE_EXP = 0.4
NORM_BATCH = True  # batched (1 recip + 1 bcast-mul per bank) vs per-q_sub

# Schraudolph constants: exp(s*0.125) ~= bf16_bits(round(s*A + B))
_SCH_A = 0.125 * 1.4426950408889634 * 128.0
_SCH_B = 16256.0 - 4.8


def _pass_plan(causal, pass_qs):
    """For one pass: list of (kb, pieces) where pieces are (qlo, qhi) column
    ranges (multiples of 128) covering q in [pass_qs[0]*128, pass_qs[-1]*128+128)
    with causal q >= 128*kb, split into <=512-wide pieces."""
    q_lo = pass_qs[0] * 128
    q_hi = pass_qs[-1] * 128 + 128
    plan = []
    for kb in range(KB):
        # align piece starts down to 512 so every piece is a contiguous
        # [128, 2, 512] tile (sub-diagonal columns are computed+exp'd but
        # never consumed by PV)
        start = max(q_lo, (128 * kb) // 512 * 512) if causal else q_lo
        if causal and 128 * kb >= q_hi:
            continue
        if start >= q_hi:
            continue
        pieces = []
        c = start
        while c < q_hi:
            ce = min(c + 512, q_hi)
            pieces.append((c, ce))
            c = ce
        plan.append((kb, pieces))
    return plan


def _emit(tc, nc, mybir, qt, kt, vg, tri, o2, causal, reps=1):
    from contextlib import ExitStack, nullcontext

    flags = _MODE_FLAGS[MODE]
    f32 = mybir.dt.float32
    bf = mybir.dt.bfloat16
    i16 = mybir.dt.int16
    Exp = mybir.ActivationFunctionType.Exp
    Mult = mybir.AluOpType.mult
    Add = mybir.AluOpType.add

    with ExitStack() as ctx:
        const = ctx.enter_context(tc.tile_pool(name="const", bufs=1))
        qk = ctx.enter_context(tc.tile_pool(name="qk", bufs=3))
        vpool = ctx.enter_context(tc.tile_pool(name="vp", bufs=3))
        ptp = ctx.enter_context(tc.tile_pool(name="ptp", bufs=4))
        outp = ctx.enter_context(tc.tile_pool(name="outp", bufs=6))
        small = ctx.enter_context(tc.tile_pool(name="small", bufs=6))
        stp = ctx.enter_context(tc.tile_pool(name="stp", bufs=3, space="PSUM"))
        accp = ctx.enter_context(tc.tile_pool(name="accp", bufs=1, space="PSUM"))

        tri_t = const.tile([128, 128], bf, name="tri_t")
        nc.sync.dma_start(tri_t[:, :], tri[:, :])

        warm = const.tile([128, 1], f32, name="warm")
        nc.vector.memset(warm[:, :], 0.0)
        nc.scalar.activation(warm[:, :], warm[:, :], Exp)

        if "d" not in flags:
            qt_c = const.tile([128, S], bf, name="qt_const")
            kt_c = const.tile([128, S], bf, name="kt_const")
            vg_c = const.tile([128, KB, 65], bf, name="vg_const")
            nc.vector.memset(qt_c[:, :], 0.0)
            nc.vector.memset(kt_c[:, :], 0.0)
            nc.vector.memset(vg_c[:, :, :], 0.0)
        if "q" not in flags and "e" in flags:
            cpsum = ctx.enter_context(
                tc.tile_pool(name="cpsum", bufs=1, space="PSUM"))
            st_cA = cpsum.tile([128, 2, 512], f32, name="st_constA")
            st_cB = cpsum.tile([128, 2, 512], f32, name="st_constB")
            nc.vector.memset(st_cA[:, :, :], 0.0)
            nc.vector.memset(st_cB[:, :, :], 0.0)
            st_consts = [st_cA, st_cB]
            st_ctr = [0]

        # DVE/ACT exp assignment: DVE only on odd piece indices so the two
        # engines always read different PSUM bank-pairs (st slot parity)
        dve_acc = [0.0]

        def use_dve():
            if DVE_EXP <= 0.0 or "e" not in flags:
                return False
            dve_acc[0] += DVE_EXP
            if dve_acc[0] >= 1.0:
                dve_acc[0] -= 1.0
                return True
            return False

        with (tc.For_i(0, reps, 1) if reps > 1 else nullcontext()):
          rep = 0
          tiles = {}
          for p in range(HEADS_PER_CORE // 2):
              # all input loads at the top of the rep on the SP queue (kept
              # free of output DMAs so the next rep's loads prefetch early);
              # chunked so pair0's first QK unblocks quickly
              if "d" in flags:
                  qt_t = qk.tile([128, S], bf, tag="qt", name=f"qt_{p}")
                  kt_t = qk.tile([128, S], bf, tag="kt", name=f"kt_{p}")
                  vg_t0 = vpool.tile([128, KB, 65], bf, tag="vg0",
                                     name=f"vg_{p}_0")
                  vg_t1 = vpool.tile([128, KB, 65], bf, tag="vg1",
                                     name=f"vg_{p}_1")
                  nc.sync.dma_start(kt_t[:, :128], kt[p][:, :128])
                  nc.sync.dma_start(qt_t[:, :512], qt[p][:, :512])
                  nc.sync.dma_start(kt_t[:, 128:], kt[p][:, 128:])
                  nc.sync.dma_start(qt_t[:, 512:1024], qt[p][:, 512:1024])
                  nc.sync.dma_start(vg_t0[:, :, :], vg[2 * p])
                  nc.sync.dma_start(vg_t1[:, :, :], vg[2 * p + 1])
                  nc.sync.dma_start(qt_t[:, 1024:], qt[p][:, 1024:])
              else:
                  qt_t, kt_t = qt_c, kt_c
                  vg_t0 = vg_t1 = vg_c
              tiles[p] = (qt_t, kt_t, (vg_t0, vg_t1))
          segs = []
          for p in range(HEADS_PER_CORE // 2):
              for pa in range(4):
                  pqs = list(range(4 * pa, 4 * pa + 4))
                  segs.append((p, pa, pqs, _pass_plan(causal, pqs)))
          gsteps = [(si, st) for si, (_, _, _, pl) in enumerate(segs)
                    for st in range(len(pl))]

          def emit_qk_g(si, step):
              p_, pa_, _, plan_ = segs[si]
              kb, pieces = plan_[step]
              qt_g, kt_g, _ = tiles[p_]
              sts = []
              for (qlo, qhi) in pieces:
                  if "q" not in flags:
                      if "e" in flags:
                          st_ctr[0] += 1
                          sts.append(st_consts[st_ctr[0] % 2])
                      else:
                          sts.append(None)
                      continue
                  st = stp.tile([128, 2, 512], f32, tag="st",
                                name=f"st_{p_}_{pa_}_{kb}_{qlo}")
                  w = qhi - qlo
                  for h in range(2):
                      po = 64 * h
                      nc.tensor.matmul(
                          st[:, h, :w],
                          lhsT=kt_g[po:po + 64, kb * 128:(kb + 1) * 128],
                          rhs=qt_g[po:po + 64, qlo:qhi],
                          start=True, stop=True,
                      )
                  sts.append(st)
              return sts

          # global 2-ahead QK window carried ACROSS pass and pair boundaries
          sts_q = [emit_qk_g(*gsteps[0]), emit_qk_g(*gsteps[1])]
          gctr = [0]
          for si in range(len(segs)):
              p, pa, pass_qs, plan = segs[si]
              qt_t, kt_t, vg_ts = tiles[p]

              if True:
                  nsub = len(pass_qs)
                  # accs: flat (h, qs_idx) -> 65-wide accumulator; 7 per bank
                  nacc = 2 * nsub
                  bank_of = lambda fl: fl // nsub
                  accs = [accp.tile([128, nsub, 65], f32,
                                    tag=f"acc{b}", name=f"acc_{p}_{pa}_{b}")
                          for b in range(2)]

                  def acc(h, qi):
                      return accs[h][:, qi, :]

                  # prepass: per-bank first/last PV matmul (emission order)
                  pv_seq = []  # (kb, piece_idx, h, qi)
                  for kb, pieces in plan:
                      for pi, (qlo, qhi) in enumerate(pieces):
                          for h in range(2):
                              qs_list = []
                              for q_sub in range(qlo // 128, qhi // 128):
                                  if causal and q_sub < kb:
                                      continue
                                  qs_list.append(q_sub)
                              if causal and kb > 0 and qs_list \
                                      and qs_list[0] == kb:
                                  qs_list = qs_list[1:] + [kb]
                              for q_sub in qs_list:
                                  pv_seq.append(
                                      (kb, pi, h, q_sub - pass_qs[0]))
                  first_in_bank = {}
                  last_in_bank = {}
                  for idx, (kb, pi, h, qi) in enumerate(pv_seq):
                      b = bank_of(h * nsub + qi)
                      first_in_bank.setdefault(b, idx)
                      last_in_bank[b] = idx
                  first_set = set(first_in_bank.values())
                  last_set = set(last_in_bank.values())

                  # bank -> kb at which it completes (for normalize timing)
                  bank_done_at = {}
                  for idx, (kb, pi, h, qi) in enumerate(pv_seq):
                      if idx in last_set:
                          bank_done_at.setdefault(kb, []).append(
                              bank_of(h * nsub + qi))

                  def emit_qk(kb, pieces):
                      sts = []
                      for (qlo, qhi) in pieces:
                          if "q" not in flags:
                              if "e" in flags:
                                  st_ctr[0] += 1
                                  sts.append(st_consts[st_ctr[0] % 2])
                              else:
                                  sts.append(None)
                              continue
                          st = stp.tile([128, 2, 512], f32, tag="st",
                                        name=f"st_{p}_{pa}_{kb}_{qlo}")
                          w = qhi - qlo
                          for h in range(2):
                              po = 64 * h
                              nc.tensor.matmul(
                                  st[:, h, :w],
                                  lhsT=kt_t[po:po + 64,
                                            kb * 128:(kb + 1) * 128],
                                  rhs=qt_t[po:po + 64, qlo:qhi],
                                  start=True, stop=True,
                              )
                          sts.append(st)
                      return sts

                  def normalize_banks(banks):
                      if "n" not in flags:
                          return
                      for b in banks:
                          fls = [fl for fl in range(nacc) if bank_of(fl) == b]
                          # group by head: contiguous q ranges per head
                          for h in range(2):
                              qis = sorted(fl - h * nsub for fl in fls
                                           if fl // nsub == h)
                              if not qis:
                                  continue
                              qs_lo = pass_qs[0] + qis[0]
                              n = len(qis)
                              ot = outp.tile([128, n, 64], bf,
                                             tag=f"ot{b}_{h}",
                                             name=f"ot_{p}_{pa}_{b}_{h}")
                              if NORM_BATCH:
                                  # one reciprocal over the bank's rowsum
                                  # column + one broadcast multiply
                                  at = accs[b]
                                  j0 = qis[0]
                                  rs = small.tile([128, n], f32,
                                                  tag=f"rs{b}_{h}",
                                                  name=f"rs_{p}_{pa}_{b}_{h}")
                                  nc.vector.reciprocal(
                                      rs[:, :], at[:, j0:j0 + n, 64])
                                  nc.vector.tensor_tensor(
                                      ot[:, :, :], at[:, j0:j0 + n, :64],
                                      rs[:, :].unsqueeze(2).broadcast_to(
                                          [128, n, 64]),
                                      mybir.AluOpType.mult)
                              else:
                                  for j, qi in enumerate(qis):
                                      a = acc(h, qi)
                                      rs = small.tile(
                                          [128, 1], f32, tag="rs",
                                          name=f"rs_{p}_{pa}_{b}_{h}_{j}")
                                      nc.vector.reciprocal(rs[:, :],
                                                           a[:, 64:65])
                                      nc.vector.tensor_scalar_mul(
                                          ot[:, j, :], a[:, :64], rs[:, :])
                              h_g = 2 * p + h
                              nc.gpsimd.dma_start(
                                  o2[h_g, :, qs_lo:qs_lo + n, :],
                                  ot[:, :, :])

                  for step, (kb, pieces) in enumerate(plan):
                      sts = sts_q.pop(0)
                      q0 = 128 * kb if causal else 0
                      # exp (ACT or DVE per piece, merged across the 2 heads)
                      pts = []
                      for (qlo, qhi), st in zip(pieces, sts):
                          if "e" not in flags:
                              pts.append(None)
                              continue
                          w = qhi - qlo
                          pt = ptp.tile([128, 2, 512], bf, tag="pt",
                                        name=f"pt_{p}_{pa}_{kb}_{qlo}")
                          if use_dve():
                              nc.vector.tensor_scalar(
                                  pt[:, :, :w].bitcast(i16), st[:, :, :w],
                                  _SCH_A, _SCH_B, Mult, Add)
                          else:
                              nc.scalar.activation(pt[:, :, :w], st[:, :, :w],
                                                   Exp, scale=0.125)
                          if causal and qlo <= q0 < qhi:
                              # intra-block mask on the diagonal 128 cols
                              dg = q0 - qlo
                              for h in range(2):
                                  nc.gpsimd.tensor_mul(pt[:, h, dg:dg + 128],
                                                       pt[:, h, dg:dg + 128],
                                                       tri_t[:, :])
                          pts.append(pt)
                      # QK two strips ahead (globally, across segments)
                      # keeps PE busy under exp without blocking on the
                      # current strip's exp (3 st slots)
                      g = gctr[0]
                      gctr[0] += 1
                      if g + 2 < len(gsteps):
                          sts_q.append(emit_qk_g(*gsteps[g + 2]))
                      # forward PV for strip kb
                      if "p" in flags and "e" in flags:
                          for idx, (kb2, pi, h, qi) in enumerate(pv_seq):
                              if kb2 != kb:
                                  continue
                              qlo, qhi = pieces[pi]
                              pt = pts[pi]
                              q_sub = pass_qs[0] + qi
                              m = q_sub * 128 - qlo
                              nc.tensor.matmul(
                                  acc(h, qi),
                                  lhsT=pt[:, h, m:m + 128],
                                  rhs=vg_ts[h][:, kb, :],
                                  start=(idx in first_set),
                                  stop=(idx in last_set),
                              )
                          normalize_banks(bank_done_at.get(kb, []))


def build_nc(causal=True, reps=1):
    key = ("nc3", causal, reps, MODE, DVE_EXP, NORM_BATCH)
    if key in _built:
        return _built[key]
    import concourse.bacc as bacc
    from concourse import mybir, tile

    nc = bacc.Bacc("TRN2", target_bir_lowering=False, debug=False,
                   num_devices=N_CORES)
    qt = nc.dram_tensor("qt", (HEADS_PER_CORE // 2, 128, S),
                        mybir.dt.bfloat16, kind="ExternalInput").ap()
    kt = nc.dram_tensor("kt", (HEADS_PER_CORE // 2, 128, S),
                        mybir.dt.bfloat16, kind="ExternalInput").ap()
    vg = nc.dram_tensor("vg", (HEADS_PER_CORE, 128, KB, 65),
                        mybir.dt.bfloat16, kind="ExternalInput").ap()
    tri = nc.dram_tensor("tri", (128, 128), mybir.dt.bfloat16,
                         kind="ExternalInput").ap()
    # output in [head, r, j, c] layout, q = j*128 + r (big DMA descriptors)
    o2 = nc.dram_tensor("o2", (HEADS_PER_CORE, 128, KB, D), mybir.dt.bfloat16,
                        kind="ExternalOutput").ap()
    with tile.TileContext(nc) as tc:
        _emit(tc, nc, mybir, qt, kt, vg, tri, o2, causal, reps)
    nc.compile()
    _built[key] = nc
    return nc


def prep_inputs(Q, K, V):
    Qf = np.ascontiguousarray(Q, dtype=np.float32).reshape(B * H, S, D)
    Kf = np.ascontiguousarray(K, dtype=np.float32).reshape(B * H, S, D)
    Vf = np.ascontiguousarray(V, dtype=np.float32).reshape(B * H, S, D)

    Qt = np.ascontiguousarray(Qf.transpose(0, 2, 1)).astype(_BF16)
    Kt = np.ascontiguousarray(Kf.transpose(0, 2, 1)).astype(_BF16)

    Vb = Vf.astype(_BF16)
    vg_all = np.empty((B * H, 128, KB, 65), dtype=_BF16)
    vg_all[:, :, :, :64] = Vb.reshape(B * H, KB, 128, D).transpose(0, 2, 1, 3)
    vg_all[:, :, :, 64] = _BF16(1.0)

    tri_np = (np.tril(np.ones((128, 128), dtype=np.float32))
              .T.astype(_BF16))
    tri_np = np.ascontiguousarray(tri_np)

    in_maps = []
    for c in range(N_CORES):
        h0 = c * HEADS_PER_CORE
        qt_c = np.empty((HEADS_PER_CORE // 2, 128, S), dtype=_BF16)
        kt_c = np.empty((HEADS_PER_CORE // 2, 128, S), dtype=_BF16)
        for p in range(HEADS_PER_CORE // 2):
            qt_c[p, :64] = Qt[h0 + 2 * p]
            qt_c[p, 64:] = Qt[h0 + 2 * p + 1]
            kt_c[p, :64] = Kt[h0 + 2 * p]
            kt_c[p, 64:] = Kt[h0 + 2 * p + 1]
        in_maps.append({
            "qt": qt_c,
            "kt": kt_c,
            "vg": np.ascontiguousarray(vg_all[h0:h0 + HEADS_PER_CORE]),
            "tri": tri_np,
        })
    return in_maps


def _classify_mask(mask):
    m = np.asarray(mask).reshape(S, S)
    if not m.any():
        return "dense"
    if np.array_equal(m, np.triu(np.ones((S, S), dtype=bool), k=1)):
        return "causal"
    raise NotImplementedError("only causal or all-False masks supported")


def run_cores(in_maps, causal=True, reps=1, **kwargs):
    from concourse import bass_utils

    nc = build_nc(causal, reps)
    return bass_utils.run_bass_kernel_spmd(
        nc, in_maps, core_ids=list(range(N_CORES)), **kwargs
    )


def kernel(Q, K, V, mask):
    kind = _classify_mask(mask)
    in_maps = prep_inputs(Q, K, V)
    res = run_cores(in_maps, causal=(kind == "causal"))
    outs = []
    for r in res.results:
        o2 = np.asarray(r["o2"], dtype=np.float32)  # [4, 128, 16, 64]
        o = o2.transpose(0, 2, 1, 3).reshape(HEADS_PER_CORE, S, D)
        outs.append(o)
    out = np.concatenate(outs, axis=0)
    return np.ascontiguousarray(out.reshape(B, H, S, D), dtype=np.float32)


if __name__ == "__main__":
    rng = np.random.default_rng(0)
    Q = rng.standard_normal((B, H, S, D), dtype=np.float32)
    K = rng.standard_normal((B, H, S, D), dtype=np.float32)
    V = rng.standard_normal((B, H, S, D), dtype=np.float32)
    mask = np.triu(np.ones((S, S), dtype=bool), k=1)[None, None]
    out = kernel(Q, K, V, mask)
    print("out", out.shape, out.dtype)
